# revision 1
# baseline (speedup 1.0000x reference)
"""EvaAdapterAttention Trainium2 kernel.

Data-parallel over batch: 32 items -> 8 cores x 4 items. Everything runs in
"feature-on-partition" (transposed) layout on-device:

  host folds the LoRA adapter into an effective qkv weight
      x' = x @ (I + (B A))^T + c   feeds ONLY qkv, so
      qkv = x @ W_eff^T + b_eff,  W_eff = qkv_w (I + B A), b_eff = qkv_w c + qkv_b
  qT/kT computed as [dim, tok] tiles (W^T stationary, x^T moving)
  v computed in natural [tok, dim] layout (x^T stationary) with an appended
      ones column per head -> attn@v also yields the softmax denominator
  scores computed transposed: S~[m, n] = sum_d k[d,m] q[d,n] (+ rel-pos bias
      preloaded into PSUM via a bf16 identity matmul), softmax without
      max-subtraction (logits are small), exp on ScalarE
  out^T[d, n] = v_aug^T @ exp(S~), normalized by broadcast reciprocal
  proj in natural layout: YT tiles are the stationary operand.

All matmuls bf16 with fp32 PSUM accumulation.
"""

import sys

if "/opt/trn_rl_repo" not in sys.path:
    sys.path.insert(0, "/opt/trn_rl_repo")

from contextlib import ExitStack

import ml_dtypes
import numpy as np

import concourse.mybir as mybir
import concourse.tile as tile
from concourse import bacc
from concourse.bass_utils import run_bass_kernel_spmd
from concourse.masks import make_identity

DIM = 1024
NUM_HEADS = 16
HEAD_DIM = 64
N = 577          # tokens (24*24 + 1 cls)
B = 32
N_CORES = 8
ITEMS = B // N_CORES          # 4 items per core
KT = DIM // 128               # 8 k-tiles
BF = mybir.dt.bfloat16
F32 = mybir.dt.float32

# token chunks of <=128 (partition dim) and matmul free-dim chunks of <=512
MCH = [(i * 128, min(128, N - i * 128)) for i in range(5)]
NCH = [(0, 512), (512, 65)]
CCH = [(0, 512), (512, 512)]  # 1024-wide free dim


def _build_program(reps: int = 1):
    nc = bacc.Bacc("TRN2", target_bir_lowering=False, debug=False)

    xT_d = nc.dram_tensor("xT", [ITEMS, DIM, N], BF, kind="ExternalInput").ap()
    wqk_d = nc.dram_tensor("wqkT", [DIM, 2 * DIM], BF, kind="ExternalInput").ap()
    wv_d = nc.dram_tensor("wvT", [DIM, DIM], BF, kind="ExternalInput").ap()
    wp_d = nc.dram_tensor("wpT", [DIM, DIM], BF, kind="ExternalInput").ap()
    qkb_d = nc.dram_tensor("qkb", [128, 16], F32, kind="ExternalInput").ap()
    vb_d = nc.dram_tensor("vb", [1, DIM], BF, kind="ExternalInput").ap()
    pb_d = nc.dram_tensor("pb", [1, DIM], BF, kind="ExternalInput").ap()
    ones_d = nc.dram_tensor("ones", [1, 128], BF, kind="ExternalInput").ap()
    bias_d = nc.dram_tensor("ebias", [NUM_HEADS, N, N], BF, kind="ExternalInput").ap()
    out_d = nc.dram_tensor("out", [ITEMS, N, DIM], F32, kind="ExternalOutput").ap()

    with tile.TileContext(nc) as tc, ExitStack() as ctx:
        wpool = ctx.enter_context(tc.tile_pool(name="weights", bufs=1))
        xpool = ctx.enter_context(tc.tile_pool(name="x", bufs=2))
        qkpool = ctx.enter_context(tc.tile_pool(name="qk", bufs=2))
        vpool = ctx.enter_context(tc.tile_pool(name="vau", bufs=2))
        ytpool = ctx.enter_context(tc.tile_pool(name="yt", bufs=2))
        bpool = ctx.enter_context(tc.tile_pool(name="bias", bufs=10))
        epool = ctx.enter_context(tc.tile_pool(name="et", bufs=3))
        npool = ctx.enter_context(tc.tile_pool(name="norm", bufs=2))
        outpool = ctx.enter_context(tc.tile_pool(name="outst", bufs=2))
        mmps = ctx.enter_context(tc.tile_pool(name="mmps", bufs=3, space="PSUM"))
        oaps = ctx.enter_context(tc.tile_pool(name="oaps", bufs=1, space="PSUM"))

        ident = wpool.tile([128, 128], BF)
        make_identity(nc, ident[:])

        wqk = [wpool.tile([128, 2 * DIM], BF, name=f"wqk{k}") for k in range(KT)]
        wv = [wpool.tile([128, DIM], BF, name=f"wv{k}") for k in range(KT)]
        wp = [wpool.tile([128, DIM], BF, name=f"wp{k}") for k in range(KT)]
        for k in range(KT):
            nc.sync.dma_start(wqk[k][:], wqk_d[k * 128 : (k + 1) * 128, :])
            nc.sync.dma_start(wv[k][:], wv_d[k * 128 : (k + 1) * 128, :])
            nc.sync.dma_start(wp[k][:], wp_d[k * 128 : (k + 1) * 128, :])
        qkb = wpool.tile([128, 16], F32)
        nc.sync.dma_start(qkb[:], qkb_d[:])
        vb = wpool.tile([1, DIM], BF)
        nc.sync.dma_start(vb[:], vb_d[:])
        pb = wpool.tile([1, DIM], BF)
        nc.sync.dma_start(pb[:], pb_d[:])
        ones = wpool.tile([1, 128], BF)
        nc.sync.dma_start(ones[:], ones_d[:])

        # per-item state (rotates through double-buffered pools)
        state = {}

        def load_x(it):
            xt = [
                xpool.tile([128, N], BF, name=f"xt{k}", tag=f"xt{k}")
                for k in range(KT)
            ]
            for k in range(KT):
                nc.sync.dma_start(xt[k][:], xT_d[it, k * 128 : (k + 1) * 128, :])
            qkT = [
                qkpool.tile([128, N], BF, name=f"qkT{t}", tag=f"qkT{t}")
                for t in range(16)
            ]
            v_aug = [
                vpool.tile([128, 16 * 65], BF, name=f"vaug{j}", tag=f"vaug{j}")
                for j in range(5)
            ]
            yt = [
                ytpool.tile([128, N], BF, name=f"yt{k}", tag=f"yt{k}")
                for k in range(KT)
            ]
            state[it] = dict(xt=xt, qkT=qkT, v_aug=v_aug, yt=yt)

        def emit_qk_tile(it, t):
            st = state[it]
            ps = mmps.tile([128, 1024], F32, name="ps", tag="mm")
            for k in range(KT):
                for n0, nn in NCH:
                    nc.tensor.matmul(
                        ps[:, n0 : n0 + nn],
                        wqk[k][:, t * 128 : (t + 1) * 128],
                        st["xt"][k][:, n0 : n0 + nn],
                        start=(k == 0), stop=(k == KT - 1),
                    )
            nc.vector.tensor_scalar(
                out=st["qkT"][t][:], in0=ps[:, 0:N], scalar1=qkb[:, t : t + 1],
                scalar2=None, op0=mybir.AluOpType.add,
            )

        def emit_v_chunk(it, j):
            st = state[it]
            m0, mc = MCH[j]
            ps = mmps.tile([128, 1024], F32, name="ps", tag="mm")
            for k in range(KT):
                for c0, cn in CCH:
                    nc.tensor.matmul(
                        ps[:mc, c0 : c0 + cn],
                        st["xt"][k][:, m0 : m0 + mc],
                        wv[k][:, c0 : c0 + cn],
                        start=(k == 0), stop=False,
                    )
            for c0, cn in CCH:
                nc.tensor.matmul(
                    ps[:mc, c0 : c0 + cn], ones[:, :mc], vb[:, c0 : c0 + cn],
                    start=False, stop=True,
                )
            va3 = st["v_aug"][j][:mc, :].rearrange("p (h c) -> p h c", h=16)
            nc.vector.memset(va3[:, :, 64:65], 1.0)
            nc.vector.tensor_copy(
                va3[:, :, 0:64],
                ps[:mc, 0:1024].rearrange("p (h c) -> p h c", h=16),
            )

        def emit_head(it, h):
            st = state[it]
            qt = st["qkT"][h // 2]
            kt_ = st["qkT"][8 + h // 2]
            r0 = 64 * (h % 2)
            bt = [
                bpool.tile([128, N], BF, name=f"bt{i}", tag="bt") for i in range(5)
            ]
            for j, (m0, mc) in enumerate(MCH):
                nc.sync.dma_start(bt[j][:mc, :], bias_d[h, m0 : m0 + mc, :])
            oa = oaps.tile([65, 1024], F32, name="oa", tag="oa")
            for j, (m0, mc) in enumerate(MCH):
                ps = mmps.tile([128, 1024], F32, name="ps", tag="mm")
                for n0, nn in NCH:
                    nc.tensor.matmul(
                        ps[:mc, n0 : n0 + nn],
                        kt_[r0 : r0 + 64, m0 : m0 + mc],
                        qt[r0 : r0 + 64, n0 : n0 + nn],
                        start=True, stop=True,
                    )
                er = epool.tile([128, N], BF, tag="er")
                nc.scalar.activation(
                    er[:mc, :], ps[:mc, 0:N], mybir.ActivationFunctionType.Exp
                )
                et = epool.tile([128, N], BF, tag="et")
                nc.vector.tensor_mul(et[:mc, :], er[:mc, :], bt[j][:mc, :])
                for n0, nn in NCH:
                    nc.tensor.matmul(
                        oa[:, n0 : n0 + nn],
                        st["v_aug"][j][:mc, h * 65 : h * 65 + 65],
                        et[:mc, n0 : n0 + nn],
                        start=(j == 0), stop=(j == 4),
                    )
            osb = npool.tile([64, N], F32, tag="osb")
            nc.vector.tensor_copy(osb[:], oa[0:64, 0:N])
            den = npool.tile([1, N], F32, tag="den")
            nc.scalar.copy(den[:], oa[64:65, 0:N])
            rec = npool.tile([1, N], F32, tag="rec")
            nc.vector.reciprocal_approx_fast(rec[:], den[:])
            rbc = npool.tile([64, N], F32, tag="rbc")
            nc.gpsimd.partition_broadcast(rbc[:], rec[:])
            nc.vector.tensor_mul(st["yt"][h // 2][r0 : r0 + 64, :], osb[:], rbc[:])

        def emit_proj_chunk(it, j):
            st = state[it]
            m0, mc = MCH[j]
            ps = mmps.tile([128, 1024], F32, name="ps", tag="mm")
            for k in range(KT):
                for c0, cn in CCH:
                    nc.tensor.matmul(
                        ps[:mc, c0 : c0 + cn],
                        st["yt"][k][:, m0 : m0 + mc],
                        wp[k][:, c0 : c0 + cn],
                        start=(k == 0), stop=False,
                    )
            for c0, cn in CCH:
                nc.tensor.matmul(
                    ps[:mc, c0 : c0 + cn], ones[:, :mc], pb[:, c0 : c0 + cn],
                    start=False, stop=True,
                )
            ob = outpool.tile([128, DIM], F32, tag="ob")
            nc.vector.tensor_copy(ob[:mc, :], ps[:mc, 0:1024])
            nc.sync.dma_start(out_d[it, m0 : m0 + mc, :], ob[:mc, :])

        # software-pipelined emission: attention(it) interleaved with
        # qkv/v(it+1) and proj(it-1)
        items = [i % ITEMS for i in range(reps * ITEMS)]
        load_x(0)
        for t in range(16):
            emit_qk_tile(0, t)
        for j in range(5):
            emit_v_chunk(0, j)
        for pos in range(len(items)):
            it = items[pos]
            gem = []
            if pos + 1 < len(items):
                nxt = items[pos + 1]
                gem += [("load", nxt, 0)]
                gem += [("qk", nxt, t) for t in range(16)]
                gem += [("v", nxt, j) for j in range(5)]
            if pos - 1 >= 0:
                gem += [("proj", items[pos - 1], j) for j in range(5)]
            # zip 16 heads with gem ops
            gi = 0
            for h in range(NUM_HEADS):
                emit_head(it, h)
                take = ((gi + len(gem) * (h + 1) // NUM_HEADS) - gi) if gem else 0
                n_take = len(gem) * (h + 1) // NUM_HEADS - gi
                for _ in range(n_take):
                    kind, which, idx = gem[gi]
                    gi += 1
                    if kind == "load":
                        load_x(which)
                    elif kind == "qk":
                        emit_qk_tile(which, idx)
                    elif kind == "v":
                        emit_v_chunk(which, idx)
                    else:
                        emit_proj_chunk(which, idx)
            state.pop(items[pos - 1], None) if pos - 1 >= 0 else None
        # final proj
        for j in range(5):
            emit_proj_chunk(items[-1], j)

    nc.compile()
    return nc


_NC = None


def _get_program(reps: int = 1):
    global _NC
    if reps != 1:
        return _build_program(reps)
    if _NC is None:
        _NC = _build_program()
    return _NC


def _prep_inputs(x, adapter_a_w, adapter_a_b, adapter_b_w, adapter_b_b,
                 qkv_w, q_bias, v_bias, rel_pos_table, proj_w, proj_b,
                 rel_pos_index):
    x = np.asarray(x, np.float32)
    A = np.asarray(adapter_a_w, np.float32)        # [8, 1024]
    a_b = np.asarray(adapter_a_b, np.float32)      # [8]
    Bw = np.asarray(adapter_b_w, np.float32)       # [1024, 8]
    b_b = np.asarray(adapter_b_b, np.float32)      # [1024]
    qkv_w = np.asarray(qkv_w, np.float32)          # [3072, 1024]
    q_bias = np.asarray(q_bias, np.float32)
    v_bias = np.asarray(v_bias, np.float32)
    table = np.asarray(rel_pos_table, np.float32)  # [2212, 16]
    proj_w = np.asarray(proj_w, np.float32)
    proj_b = np.asarray(proj_b, np.float32)
    idx = np.asarray(rel_pos_index, np.int64)      # [577, 577]

    M = np.eye(DIM, dtype=np.float32) + Bw @ A
    W_eff = qkv_w @ M                              # [3072, 1024]
    c = a_b @ Bw.T + b_b                           # [1024]
    qkv_b = np.concatenate([q_bias, np.zeros(DIM, np.float32), v_bias])
    b_eff = qkv_w @ c + qkv_b                      # [3072]

    scale = HEAD_DIM ** -0.5
    W_eff[0:DIM] *= scale
    b_eff[0:DIM] *= scale

    bf = ml_dtypes.bfloat16
    wqkT = np.ascontiguousarray(W_eff[0 : 2 * DIM].T).astype(bf)
    wvT = np.ascontiguousarray(W_eff[2 * DIM :].T).astype(bf)
    wpT = np.ascontiguousarray(proj_w.T).astype(bf)
    qkb = np.ascontiguousarray(b_eff[0 : 2 * DIM].reshape(16, 128).T).astype(
        np.float32
    )
    vb = b_eff[2 * DIM :][None].astype(bf)
    pb = proj_b[None].astype(bf)
    ones = np.ones((1, 128), bf)

    # bias_h[n, m] = table[idx[n, m], h]; device wants biasT[h, m, n]
    gathered = table[idx.reshape(-1)].reshape(N, N, NUM_HEADS)
    ebias = np.exp(np.ascontiguousarray(gathered.transpose(2, 1, 0))).astype(bf)

    xT = np.ascontiguousarray(x.transpose(0, 2, 1)).astype(bf)  # [32, 1024, 577]

    shared = {
        "wqkT": wqkT, "wvT": wvT, "wpT": wpT, "qkb": qkb, "vb": vb, "pb": pb,
        "ones": ones, "ebias": ebias,
    }
    in_maps = []
    for core in range(N_CORES):
        m_ = dict(shared)
        m_["xT"] = np.ascontiguousarray(xT[core * ITEMS : (core + 1) * ITEMS])
        in_maps.append(m_)
    return in_maps


def run_on_device(in_maps, **kwargs):
    nc = _get_program()
    return run_bass_kernel_spmd(nc, in_maps, list(range(N_CORES)), **kwargs)


def kernel(**inputs) -> np.ndarray:
    in_maps = _prep_inputs(**inputs)
    res = run_on_device(in_maps)
    out = np.concatenate([r["out"] for r in res.results], axis=0)
    return out.astype(np.float32)


if __name__ == "__main__":
    rng = np.random.RandomState(0)
    fake = {
        "x": rng.randn(B, N, DIM).astype(np.float32),
        "adapter_a_w": rng.randn(8, DIM).astype(np.float32) * 0.02,
        "adapter_a_b": np.zeros(8, np.float32),
        "adapter_b_w": rng.randn(DIM, 8).astype(np.float32) * 0.02,
        "adapter_b_b": np.zeros(DIM, np.float32),
        "qkv_w": rng.randn(3 * DIM, DIM).astype(np.float32) * 0.02,
        "q_bias": rng.randn(DIM).astype(np.float32) * 0.02,
        "v_bias": rng.randn(DIM).astype(np.float32) * 0.02,
        "rel_pos_table": rng.randn(2212, NUM_HEADS).astype(np.float32) * 0.02,
        "proj_w": rng.randn(DIM, DIM).astype(np.float32) * 0.02,
        "proj_b": np.zeros(DIM, np.float32),
        "rel_pos_index": rng.randint(0, 2212, (N, N)).astype(np.int32),
    }
    out = kernel(**fake)
    print("out", out.shape, out.dtype, np.abs(out).max())

